# revision 11
# baseline (speedup 1.0000x reference)
"""DistanceLoss kernel for 8 Trainium2 NeuronCores.

Reference computation (T=64, H=32, W=8, B=2048):
    belongs = target.T                              # [T, B] in {0,1}
    iwd  = sum_w inner_window_distances             # [T, H, B]
    cow  = sum_w outer_window_distances             # [T, H, B]
    bl   = belongs*(1-cont)*(ofd + iwd)             # [T, H, B]
    nbl  = (1-belongs)*cont*(ifd + cow)             # [T, H, B]
    loss = mean_b sum_t [ min_h bl + max_h nbl ]

c1 = belongs*(1-cont), c2 = (1-belongs)*cont are {0,1} and constant over h,
so min_h bl == c1 * min_h(ofd+iwd), max_h nbl == c2 * max_h(ifd+cow).

Sharding: T split 8 ways (8 towns/core), contiguous per-core DRAM slabs.

Dataflow per (side, th, bh) iteration (bf16):
  1. two SWDGE cast-DMAs f32->bf16 [128=(t4 h32), (w4 b1024)]
     (HBM read at ~420 GB/s; write side halves -> same read roofline)
  2. bf16 DVE tree (2x mode): 6 adds -> a[128, 1024] incl. frame add
  3. PE transposes (bf16, full rate) 128x128 blocks -> PSUM
  4. DVE min/max reduce over h per PSUM bank -> m1/m2 (f32)
  5. masks: host-packed dense [128, (bc t)] f32 bel/cT -> c1/c2 on DVE
     AFTER side 0 (keeps the in-order DVE queue head free; v4 lost
     40us to mask DMAs with 8B descriptors starving behind the big
     stream while DVE head-of-line blocked on them)
  6. z[p, bc] partial loss -> host sums cores, means.
"""

import numpy as np

T, H, W, B = 64, 32, 8, 2048
NCORES = 8
TL = T // NCORES          # 8 local towns per core
NBC = B // 128            # 16 batch chunks of 128

_CACHE = {}


def _build_program():
    import concourse.tile as tile
    from concourse import bacc, mybir

    f32 = mybir.dt.float32
    bf16 = mybir.dt.bfloat16
    AX = mybir.AxisListType
    OP = mybir.AluOpType

    nc = bacc.Bacc()
    iw = nc.declare_dram_parameter("iw", [TL, H, W, B], f32, isOutput=False)
    ow = nc.declare_dram_parameter("ow", [TL, H, W, B], f32, isOutput=False)
    ofd = nc.declare_dram_parameter("ofd", [TL, H, B], f32, isOutput=False)
    ifd = nc.declare_dram_parameter("ifd", [TL, H, B], f32, isOutput=False)
    # host-packed dense masks: [p, (bc t)] f32, b = bc*128 + p
    belP = nc.declare_dram_parameter("belP", [128, NBC * TL], f32, isOutput=False)
    cTP = nc.declare_dram_parameter("cTP", [128, NBC * TL], f32, isOutput=False)
    z = nc.declare_dram_parameter("z", [128, NBC], f32, isOutput=True)

    ident = nc.inline_tensor(np.eye(128, dtype=np.float32), name="ident128")

    BH = B // 2

    with tile.TileContext(nc) as tc:
        with (
            tc.tile_pool(name="const", bufs=1) as const_pool,
            tc.tile_pool(name="big", bufs=8) as big_pool,
            tc.tile_pool(name="frame", bufs=4) as frame_pool,
            tc.tile_pool(name="tmp", bufs=6) as tmp_pool,
            tc.tile_pool(name="atile", bufs=3) as a_pool,
            tc.tile_pool(name="mres", bufs=1) as m_pool,
            tc.tile_pool(name="fin", bufs=1) as fin_pool,
            tc.tile_pool(name="ps", bufs=8, space="PSUM") as psum_pool,
        ):
            # m1/m2: col = bc*TL + t
            m1 = m_pool.tile([128, NBC * TL], f32, tag="m1")
            m2 = m_pool.tile([128, NBC * TL], f32, tag="m2")

            frs = {}
            state = {}

            def emit_const_dmas():
                identt = const_pool.tile([128, 128], f32)
                nc.sync.dma_start(identt[:], ident[:, :])
                identc = const_pool.tile([128, 128], bf16)
                nc.vector.tensor_copy(identc[:], identt[:])
                state["identt"] = identt
                bel = fin_pool.tile([128, NBC * TL], f32, tag="bel")
                nc.sync.dma_start(bel[:], belP[:, :])
                cT = fin_pool.tile([128, NBC * TL], f32, tag="cT")
                nc.sync.dma_start(cT[:], cTP[:, :])
                state["identc"] = identc
                state["bel"] = bel
                state["cT"] = cT

            def emit_masks():
                bel, cT = state["bel"], state["cT"]
                # c1 = bel - bel*cT ; c2 = cT - bel*cT
                bc_t = fin_pool.tile([128, NBC * TL], f32, tag="bct")
                nc.vector.tensor_mul(bc_t[:], bel[:], cT[:])
                c1 = fin_pool.tile([128, NBC * TL], f32, tag="c1")
                nc.vector.tensor_sub(c1[:], bel[:], bc_t[:])
                c2 = fin_pool.tile([128, NBC * TL], f32, tag="c2")
                nc.vector.tensor_sub(c2[:], cT[:], bc_t[:])
                state["c1"] = c1
                state["c2"] = c2

            def do_iter(side, th, b0, bw):
                src4 = iw if side == 0 else ow
                src3 = ofd if side == 0 else ifd
                mdst = m1 if side == 0 else m2
                red_op = OP.min if side == 0 else OP.max
                mview = mdst[:].rearrange("p (c t) -> p c t", t=TL)
                t0 = th * 4

                first = side == 0 and th == 0 and b0 == 0
                ddt = f32 if first else bf16
                eng = nc.sync if first else nc.gpsimd
                bts = []
                for wh in range(2):
                    bt = big_pool.tile(
                        [128, 4 * bw], ddt,
                        tag=("bigf" if first else "big"),
                        name=f"bt{side}{th}{b0}{wh}",
                        bufs=(2 if first else None),
                    )
                    eng.dma_start(
                        bt[:].rearrange("p (w b) -> p w b", w=4),
                        src4[
                            t0 : t0 + 4, :, 4 * wh : 4 * wh + 4,
                            b0 : b0 + bw,
                        ].rearrange("t h w b -> (t h) w b"),
                    )
                    bts.append(bt)

                if b0 == 0:
                    fr = frame_pool.tile([128, B], bf16, tag="fr")
                    nc.gpsimd.dma_start(
                        fr[:],
                        src3[t0 : t0 + 4, :, :].rearrange("t h b -> (t h) b"),
                    )
                    frs[(side, th)] = fr

                if "identc" not in state:
                    emit_const_dmas()
                identc = state["identc"]
                fr = frs[(side, th)]

                # wide-pair tree: (w0+w2 | w1+w3) then fold, per tile
                vs = []
                for bt in bts:
                    u = tmp_pool.tile(
                        [128, 2 * bw], ddt,
                        tag=("uf" if first else "u"),
                        name=f"u{side}{th}{b0}",
                        bufs=(2 if first else None),
                    )
                    nc.vector.tensor_add(
                        u[:], bt[:, 0 : 2 * bw], bt[:, 2 * bw : 4 * bw]
                    )
                    v = tmp_pool.tile(
                        [128, bw], ddt,
                        tag=("vf" if first else "v"),
                        name=f"v{side}{th}{b0}",
                        bufs=(4 if first else None),
                    )
                    nc.vector.tensor_add(v[:], u[:, 0:bw], u[:, bw : 2 * bw])
                    vs.append(v)
                s = tmp_pool.tile(
                    [128, bw], ddt,
                    tag=("vf" if first else "v"), name=f"s{side}{th}{b0}",
                    bufs=(4 if first else None),
                )
                nc.vector.tensor_add(s[:], vs[0][:], vs[1][:])
                a = a_pool.tile(
                    [128, bw], ddt,
                    tag=("af" if first else "a"), name=f"a{side}{th}{b0}",
                    bufs=(1 if first else None),
                )
                if first:
                    frf = tmp_pool.tile([128, bw], f32, tag="vf", bufs=4)
                    nc.vector.tensor_copy(frf[:], fr[:, b0 : b0 + bw])
                    nc.vector.tensor_add(a[:], s[:], frf[:])
                else:
                    nc.vector.tensor_add(a[:], s[:], fr[:, b0 : b0 + bw])

                # PE transposes: 4 x 128x128 per PSUM bank, then one
                # batched min/max reduce per bank (512-wide groups)
                idt = state["identt"] if first else identc
                for g in range(max(1, bw // 512)):
                    gw = min(bw, 512)
                    pt = psum_pool.tile(
                        [128, 512], ddt, tag="pt", name=f"pt{side}{th}{b0}{g}"
                    )
                    for q in range(gw // 128):
                        lc = g * 4 + q
                        nc.tensor.transpose(
                            pt[:, q * 128 : (q + 1) * 128],
                            a[:, lc * 128 : (lc + 1) * 128],
                            idt[:],
                        )
                    cs = (b0 + g * 512) // 128
                    nc.vector.tensor_reduce(
                        mview[:, cs : cs + gw // 128, t0 : t0 + 4],
                        pt[:, 0:gw].rearrange(
                            "p (c t h) -> p c t h", t=4, h=H
                        ),
                        axis=AX.X,
                        op=red_op,
                    )

            def tail_iter(b0):
                """Last 512-wide chunk of (side1, th1): 4-way w-pair DMAs
                so only a short fold chain depends on the last byte."""
                t0 = 4
                mview = m2[:].rearrange("p (c t) -> p c t", t=TL)
                identc = state["identc"]
                fr = frs[(1, 1)]
                vs = []
                for wp in range(4):
                    wt_ = big_pool.tile(
                        [128, 1024], bf16, tag="big", name=f"tw{wp}"
                    )
                    nc.gpsimd.dma_start(
                        wt_[:].rearrange("p (w b) -> p w b", w=2),
                        ow[
                            t0 : t0 + 4, :, 2 * wp : 2 * wp + 2,
                            b0 : b0 + 512,
                        ].rearrange("t h w b -> (t h) w b"),
                    )
                    v = tmp_pool.tile([128, 512], bf16, tag="v", name=f"tv{wp}")
                    nc.vector.tensor_add(v[:], wt_[:, 0:512], wt_[:, 512:1024])
                    vs.append(v)
                    if wp == 1:
                        # pair-sum of the first two folds runs during the
                        # later DMAs' drain, not behind v3 in the queue
                        s1 = tmp_pool.tile([128, 512], bf16, tag="v")
                        nc.vector.tensor_add(s1[:], vs[0][:], vs[1][:])
                s2 = tmp_pool.tile([128, 512], bf16, tag="v")
                nc.vector.tensor_add(s2[:], vs[2][:], vs[3][:])
                s = tmp_pool.tile([128, 512], bf16, tag="v")
                nc.vector.tensor_add(s[:], s1[:], s2[:])
                a = a_pool.tile([128, 512], bf16, tag="a")
                nc.vector.tensor_add(a[:], s[:], fr[:, b0 : b0 + 512])
                cs = b0 // 128
                for half in range(2):
                    # separate PSUM tiles (banks) so the first half's
                    # reduce overlaps the second half's transposes
                    pt = psum_pool.tile(
                        [128, 256], bf16, tag="pt", name=f"ptt{half}"
                    )
                    for q in range(2):
                        lc = half * 2 + q
                        nc.tensor.transpose(
                            pt[:, q * 128 : (q + 1) * 128],
                            a[:, lc * 128 : (lc + 1) * 128],
                            identc[:],
                        )
                    nc.vector.tensor_reduce(
                        mview[:, cs + 2 * half : cs + 2 * half + 2, t0 : t0 + 4],
                        pt[:].rearrange("p (c t h) -> p c t h", t=4, h=H),
                        axis=AX.X,
                        op=OP.max,
                    )

            m2v = m2[:].rearrange("p (c t) -> p c t", t=TL)
            for side in range(2):
                for th in range(2):
                    if side == 1 and th == 1:
                        # shorter dependent chains after the last DMA byte
                        do_iter(side, th, 0, BH)
                        do_iter(side, th, BH, 512)
                        tail_iter(BH + 512)
                    else:
                        do_iter(side, th, 0, BH)
                        do_iter(side, th, BH, BH)
                    if side == 1 and th == 0:
                        # side-1/th-0 half of the final combine mid-stream
                        c2v = state["c2"][:].rearrange(
                            "p (c t) -> p c t", t=TL
                        )
                        w2a = fin_pool.tile([128, NBC * 4], f32, tag="w2a")
                        w2av = w2a[:].rearrange("p (c t) -> p c t", t=4)
                        nc.vector.tensor_mul(
                            w2av[:], c2v[:, :, 0:4], m2v[:, :, 0:4]
                        )
                        zb2a = fin_pool.tile([128, NBC], f32, tag="zb2a")
                        nc.vector.tensor_reduce(
                            zb2a[:],
                            w2av[:],
                            axis=AX.X,
                            op=OP.add,
                        )
                if side == 0:
                    # mask products + side-0 weighting + partial loss sum
                    emit_masks()
                    w1 = fin_pool.tile([128, NBC * TL], f32, tag="w1")
                    nc.vector.tensor_mul(w1[:], state["c1"][:], m1[:])
                    zb1 = fin_pool.tile([128, NBC], f32, tag="zb1")
                    nc.vector.tensor_reduce(
                        zb1[:],
                        w1[:].rearrange("p (c t) -> p c t", t=TL),
                        axis=AX.X,
                        op=OP.add,
                    )

            c2v = state["c2"][:].rearrange("p (c t) -> p c t", t=TL)
            w2b = fin_pool.tile([128, NBC * 4], f32, tag="w2b")
            w2bv = w2b[:].rearrange("p (c t) -> p c t", t=4)
            nc.vector.tensor_mul(w2bv[:], c2v[:, :, 4:8], m2v[:, :, 4:8])
            zb2b = fin_pool.tile([128, NBC], f32, tag="zb2b")
            nc.vector.tensor_reduce(
                zb2b[:], w2bv[:], axis=AX.X, op=OP.add
            )
            zb12 = fin_pool.tile([128, NBC], f32, tag="zb12")
            nc.vector.tensor_add(zb12[:], zb1[:], zb2a[:])
            zb = fin_pool.tile([128, NBC], f32, tag="zb")
            nc.vector.tensor_add(zb[:], zb12[:], zb2b[:])
            nc.sync.dma_start(z[:, :], zb[:])

    nc.finalize()
    return nc


def _get_program():
    if "nc" not in _CACHE:
        _CACHE["nc"] = _build_program()
    return _CACHE["nc"]


def _pack_mask(m):
    """[B, TL] -> [128, NBC*TL] f32 with out[p, c*TL+t] = m[c*128+p, t]."""
    return np.ascontiguousarray(
        m.reshape(NBC, 128, TL).transpose(1, 0, 2).reshape(128, NBC * TL)
    ).astype(np.float32)


def _make_in_maps(iw, owd, ofd, ifd, contT, tgt):
    in_maps = []
    for c in range(NCORES):
        t0, t1 = c * TL, (c + 1) * TL
        in_maps.append(
            {
                "iw": np.ascontiguousarray(iw[t0:t1]),
                "ow": np.ascontiguousarray(owd[t0:t1]),
                "ofd": np.ascontiguousarray(ofd[t0:t1]),
                "ifd": np.ascontiguousarray(ifd[t0:t1]),
                "belP": _pack_mask(tgt[:, t0:t1]),
                "cTP": _pack_mask(contT[:, t0:t1]),
            }
        )
    return in_maps


def kernel(
    inner_window_distances: np.ndarray,
    outer_window_distances: np.ndarray,
    outer_frame_distance: np.ndarray,
    inner_frame_distance: np.ndarray,
    containment: np.ndarray,
    target: np.ndarray,
) -> np.ndarray:
    from concourse.bass_utils import run_bass_kernel_spmd

    nc = _get_program()

    iw = np.ascontiguousarray(inner_window_distances, dtype=np.float32)
    owd = np.ascontiguousarray(outer_window_distances, dtype=np.float32)
    ofd = np.ascontiguousarray(outer_frame_distance, dtype=np.float32)
    ifd = np.ascontiguousarray(inner_frame_distance, dtype=np.float32)
    contT = np.ascontiguousarray(containment, dtype=np.float32).T  # [B, T]
    tgt = np.ascontiguousarray(target).view(np.uint8)

    in_maps = _make_in_maps(iw, owd, ofd, ifd, contT, tgt)
    res = run_bass_kernel_spmd(nc, in_maps, list(range(NCORES)))

    # z[p, bc] (per core) = partial loss for b = bc*128 + p, summed over
    # the core's 8 towns.  Sum cores, flatten to [B], mean.
    acc = np.zeros((128, NBC), dtype=np.float64)
    for r in res.results:
        acc += r["z"].astype(np.float64)
    loss_b = acc.T.reshape(B)
    return np.float32(loss_b.mean())


# revision 13
# speedup vs baseline: 1.0106x; 1.0106x over previous
"""DistanceLoss kernel for 8 Trainium2 NeuronCores.

Reference computation (T=64, H=32, W=8, B=2048):
    belongs = target.T                              # [T, B] in {0,1}
    iwd  = sum_w inner_window_distances             # [T, H, B]
    cow  = sum_w outer_window_distances             # [T, H, B]
    bl   = belongs*(1-cont)*(ofd + iwd)             # [T, H, B]
    nbl  = (1-belongs)*cont*(ifd + cow)             # [T, H, B]
    loss = mean_b sum_t [ min_h bl + max_h nbl ]

c1 = belongs*(1-cont), c2 = (1-belongs)*cont are {0,1} and constant over h,
so min_h bl == c1 * min_h(ofd+iwd), max_h nbl == c2 * max_h(ifd+cow).

Sharding: T split 8 ways (8 towns/core), contiguous per-core DRAM slabs.

Dataflow per (side, th, bh) iteration (bf16):
  1. two SWDGE cast-DMAs f32->bf16 [128=(t4 h32), (w4 b1024)]
     (HBM read at ~420 GB/s; write side halves -> same read roofline)
  2. bf16 DVE tree (2x mode): 6 adds -> a[128, 1024] incl. frame add
  3. PE transposes (bf16, full rate) 128x128 blocks -> PSUM
  4. DVE min/max reduce over h per PSUM bank -> m1/m2 (f32)
  5. masks: host-packed dense [128, (bc t)] f32 bel/cT -> c1/c2 on DVE
     AFTER side 0 (keeps the in-order DVE queue head free; v4 lost
     40us to mask DMAs with 8B descriptors starving behind the big
     stream while DVE head-of-line blocked on them)
  6. z[p, bc] partial loss -> host sums cores, means.
"""

import numpy as np

T, H, W, B = 64, 32, 8, 2048
NCORES = 8
TL = T // NCORES          # 8 local towns per core
NBC = B // 128            # 16 batch chunks of 128

_CACHE = {}


def _build_program():
    import concourse.tile as tile
    from concourse import bacc, mybir

    f32 = mybir.dt.float32
    bf16 = mybir.dt.bfloat16
    AX = mybir.AxisListType
    OP = mybir.AluOpType

    nc = bacc.Bacc()
    iw = nc.declare_dram_parameter("iw", [TL, H, W, B], f32, isOutput=False)
    ow = nc.declare_dram_parameter("ow", [TL, H, W, B], f32, isOutput=False)
    ofd = nc.declare_dram_parameter("ofd", [TL, H, B], f32, isOutput=False)
    ifd = nc.declare_dram_parameter("ifd", [TL, H, B], f32, isOutput=False)
    # host-packed dense masks: [p, (bc t)] f32, b = bc*128 + p
    belP = nc.declare_dram_parameter("belP", [128, NBC * TL], f32, isOutput=False)
    cTP = nc.declare_dram_parameter("cTP", [128, NBC * TL], f32, isOutput=False)
    z = nc.declare_dram_parameter("z", [128, NBC], f32, isOutput=True)

    ident = nc.inline_tensor(np.eye(128, dtype=np.float32), name="ident128")

    BH = B // 2

    with tile.TileContext(nc) as tc:
        with (
            tc.tile_pool(name="const", bufs=1) as const_pool,
            tc.tile_pool(name="big", bufs=8) as big_pool,
            tc.tile_pool(name="frame", bufs=4) as frame_pool,
            tc.tile_pool(name="tmp", bufs=6) as tmp_pool,
            tc.tile_pool(name="atile", bufs=3) as a_pool,
            tc.tile_pool(name="mres", bufs=1) as m_pool,
            tc.tile_pool(name="fin", bufs=1) as fin_pool,
            tc.tile_pool(name="ps", bufs=8, space="PSUM") as psum_pool,
        ):
            # m1/m2: col = bc*TL + t
            m1 = m_pool.tile([128, NBC * TL], f32, tag="m1")
            m2 = m_pool.tile([128, NBC * TL], f32, tag="m2")

            frs = {}
            state = {}

            def emit_const_dmas():
                identt = const_pool.tile([128, 128], f32)
                nc.sync.dma_start(identt[:], ident[:, :])
                identc = const_pool.tile([128, 128], bf16)
                nc.vector.tensor_copy(identc[:], identt[:])
                bel = fin_pool.tile([128, NBC * TL], f32, tag="bel")
                nc.sync.dma_start(bel[:], belP[:, :])
                cT = fin_pool.tile([128, NBC * TL], f32, tag="cT")
                nc.sync.dma_start(cT[:], cTP[:, :])
                state["identc"] = identc
                state["bel"] = bel
                state["cT"] = cT

            def emit_masks():
                bel, cT = state["bel"], state["cT"]
                # c1 = bel - bel*cT ; c2 = cT - bel*cT
                bc_t = fin_pool.tile([128, NBC * TL], f32, tag="bct")
                nc.vector.tensor_mul(bc_t[:], bel[:], cT[:])
                c1 = fin_pool.tile([128, NBC * TL], f32, tag="c1")
                nc.vector.tensor_sub(c1[:], bel[:], bc_t[:])
                c2 = fin_pool.tile([128, NBC * TL], f32, tag="c2")
                nc.vector.tensor_sub(c2[:], cT[:], bc_t[:])
                state["c1"] = c1
                state["c2"] = c2

            def do_iter(side, th, b0, bw):
                src4 = iw if side == 0 else ow
                src3 = ofd if side == 0 else ifd
                mdst = m1 if side == 0 else m2
                red_op = OP.min if side == 0 else OP.max
                mview = mdst[:].rearrange("p (c t) -> p c t", t=TL)
                t0 = th * 4

                bts = []
                for wh in range(2):
                    bt = big_pool.tile([128, 4 * bw], bf16, tag="big")
                    nc.gpsimd.dma_start(
                        bt[:].rearrange("p (w b) -> p w b", w=4),
                        src4[
                            t0 : t0 + 4, :, 4 * wh : 4 * wh + 4,
                            b0 : b0 + bw,
                        ].rearrange("t h w b -> (t h) w b"),
                    )
                    bts.append(bt)

                if b0 == 0:
                    fr = frame_pool.tile([128, B], bf16, tag="fr")
                    nc.gpsimd.dma_start(
                        fr[:],
                        src3[t0 : t0 + 4, :, :].rearrange("t h b -> (t h) b"),
                    )
                    frs[(side, th)] = fr

                if "identc" not in state:
                    emit_const_dmas()
                identc = state["identc"]
                fr = frs[(side, th)]

                # wide-pair tree: (w0+w2 | w1+w3) then fold, per tile
                vs = []
                for bt in bts:
                    u = tmp_pool.tile([128, 2 * bw], bf16, tag="u")
                    nc.vector.tensor_add(
                        u[:], bt[:, 0 : 2 * bw], bt[:, 2 * bw : 4 * bw]
                    )
                    v = tmp_pool.tile([128, bw], bf16, tag="v")
                    nc.vector.tensor_add(v[:], u[:, 0:bw], u[:, bw : 2 * bw])
                    vs.append(v)
                s = tmp_pool.tile([128, bw], bf16, tag="v")
                nc.vector.tensor_add(s[:], vs[0][:], vs[1][:])
                a = a_pool.tile([128, bw], bf16, tag="a")
                nc.vector.tensor_add(a[:], s[:], fr[:, b0 : b0 + bw])

                # PE transposes: 4 x 128x128 per PSUM bank, then one
                # batched min/max reduce per bank (512-wide groups)
                for g in range(max(1, bw // 512)):
                    gw = min(bw, 512)
                    pt = psum_pool.tile([128, 512], bf16, tag="pt")
                    for q in range(gw // 128):
                        lc = g * 4 + q
                        nc.tensor.transpose(
                            pt[:, q * 128 : (q + 1) * 128],
                            a[:, lc * 128 : (lc + 1) * 128],
                            identc[:],
                        )
                    cs = (b0 + g * 512) // 128
                    nc.vector.tensor_reduce(
                        mview[:, cs : cs + gw // 128, t0 : t0 + 4],
                        pt[:, 0:gw].rearrange(
                            "p (c t h) -> p c t h", t=4, h=H
                        ),
                        axis=AX.X,
                        op=red_op,
                    )

            def tail_iter(b0):
                """Last 512-wide chunk of (side1, th1): 4-way w-pair DMAs
                so only a short fold chain depends on the last byte."""
                t0 = 4
                mview = m2[:].rearrange("p (c t) -> p c t", t=TL)
                identc = state["identc"]
                fr = frs[(1, 1)]
                vs = []
                for wp in range(4):
                    wt_ = big_pool.tile(
                        [128, 1024], bf16, tag="big", name=f"tw{wp}"
                    )
                    nc.gpsimd.dma_start(
                        wt_[:].rearrange("p (w b) -> p w b", w=2),
                        ow[
                            t0 : t0 + 4, :, 2 * wp : 2 * wp + 2,
                            b0 : b0 + 512,
                        ].rearrange("t h w b -> (t h) w b"),
                    )
                    v = tmp_pool.tile([128, 512], bf16, tag="v", name=f"tv{wp}")
                    nc.vector.tensor_add(v[:], wt_[:, 0:512], wt_[:, 512:1024])
                    vs.append(v)
                    if wp == 1:
                        # pair-sum of the first two folds + the frame add
                        # both run during the later DMAs' drain; the
                        # post-arrival chain is then just s2 -> a
                        s1 = tmp_pool.tile([128, 512], bf16, tag="v")
                        nc.vector.tensor_add(s1[:], vs[0][:], vs[1][:])
                        s1f = tmp_pool.tile([128, 512], bf16, tag="v")
                        nc.vector.tensor_add(
                            s1f[:], s1[:], fr[:, b0 : b0 + 512]
                        )
                s2 = tmp_pool.tile([128, 512], bf16, tag="v")
                nc.vector.tensor_add(s2[:], vs[2][:], vs[3][:])
                a = a_pool.tile([128, 512], bf16, tag="a")
                nc.vector.tensor_add(a[:], s1f[:], s2[:])
                cs = b0 // 128
                for half in range(2):
                    # separate PSUM tiles (banks) so the first half's
                    # reduce overlaps the second half's transposes
                    pt = psum_pool.tile(
                        [128, 256], bf16, tag="pt", name=f"ptt{half}"
                    )
                    for q in range(2):
                        lc = half * 2 + q
                        nc.tensor.transpose(
                            pt[:, q * 128 : (q + 1) * 128],
                            a[:, lc * 128 : (lc + 1) * 128],
                            identc[:],
                        )
                    nc.vector.tensor_reduce(
                        mview[:, cs + 2 * half : cs + 2 * half + 2, t0 : t0 + 4],
                        pt[:].rearrange("p (c t h) -> p c t h", t=4, h=H),
                        axis=AX.X,
                        op=OP.max,
                    )

            m2v = m2[:].rearrange("p (c t) -> p c t", t=TL)
            for side in range(2):
                for th in range(2):
                    if side == 1 and th == 1:
                        # shorter dependent chains after the last DMA byte
                        do_iter(side, th, 0, BH)
                        do_iter(side, th, BH, 512)
                        tail_iter(BH + 512)
                    else:
                        do_iter(side, th, 0, BH)
                        do_iter(side, th, BH, BH)
                    if side == 1 and th == 0:
                        # side-1/th-0 half of the final combine mid-stream
                        c2v = state["c2"][:].rearrange(
                            "p (c t) -> p c t", t=TL
                        )
                        w2a = fin_pool.tile([128, NBC * 4], f32, tag="w2a")
                        w2av = w2a[:].rearrange("p (c t) -> p c t", t=4)
                        nc.vector.tensor_mul(
                            w2av[:], c2v[:, :, 0:4], m2v[:, :, 0:4]
                        )
                        zb2a = fin_pool.tile([128, NBC], f32, tag="zb2a")
                        nc.vector.tensor_reduce(
                            zb2a[:],
                            w2av[:],
                            axis=AX.X,
                            op=OP.add,
                        )
                if side == 0:
                    # mask products + side-0 weighting + partial loss sum
                    emit_masks()
                    w1 = fin_pool.tile([128, NBC * TL], f32, tag="w1")
                    nc.vector.tensor_mul(w1[:], state["c1"][:], m1[:])
                    zb1 = fin_pool.tile([128, NBC], f32, tag="zb1")
                    nc.vector.tensor_reduce(
                        zb1[:],
                        w1[:].rearrange("p (c t) -> p c t", t=TL),
                        axis=AX.X,
                        op=OP.add,
                    )

            c2v = state["c2"][:].rearrange("p (c t) -> p c t", t=TL)
            w2b = fin_pool.tile([128, NBC * 4], f32, tag="w2b")
            w2bv = w2b[:].rearrange("p (c t) -> p c t", t=4)
            nc.vector.tensor_mul(w2bv[:], c2v[:, :, 4:8], m2v[:, :, 4:8])
            zb2b = fin_pool.tile([128, NBC], f32, tag="zb2b")
            nc.vector.tensor_reduce(
                zb2b[:], w2bv[:], axis=AX.X, op=OP.add
            )
            zb12 = fin_pool.tile([128, NBC], f32, tag="zb12")
            nc.vector.tensor_add(zb12[:], zb1[:], zb2a[:])
            zb = fin_pool.tile([128, NBC], f32, tag="zb")
            nc.vector.tensor_add(zb[:], zb12[:], zb2b[:])
            nc.sync.dma_start(z[:, :], zb[:])

    nc.finalize()
    return nc


def _get_program():
    if "nc" not in _CACHE:
        _CACHE["nc"] = _build_program()
    return _CACHE["nc"]


def _pack_mask(m):
    """[B, TL] -> [128, NBC*TL] f32 with out[p, c*TL+t] = m[c*128+p, t]."""
    return np.ascontiguousarray(
        m.reshape(NBC, 128, TL).transpose(1, 0, 2).reshape(128, NBC * TL)
    ).astype(np.float32)


def _make_in_maps(iw, owd, ofd, ifd, contT, tgt):
    in_maps = []
    for c in range(NCORES):
        t0, t1 = c * TL, (c + 1) * TL
        in_maps.append(
            {
                "iw": np.ascontiguousarray(iw[t0:t1]),
                "ow": np.ascontiguousarray(owd[t0:t1]),
                "ofd": np.ascontiguousarray(ofd[t0:t1]),
                "ifd": np.ascontiguousarray(ifd[t0:t1]),
                "belP": _pack_mask(tgt[:, t0:t1]),
                "cTP": _pack_mask(contT[:, t0:t1]),
            }
        )
    return in_maps


def kernel(
    inner_window_distances: np.ndarray,
    outer_window_distances: np.ndarray,
    outer_frame_distance: np.ndarray,
    inner_frame_distance: np.ndarray,
    containment: np.ndarray,
    target: np.ndarray,
) -> np.ndarray:
    from concourse.bass_utils import run_bass_kernel_spmd

    nc = _get_program()

    iw = np.ascontiguousarray(inner_window_distances, dtype=np.float32)
    owd = np.ascontiguousarray(outer_window_distances, dtype=np.float32)
    ofd = np.ascontiguousarray(outer_frame_distance, dtype=np.float32)
    ifd = np.ascontiguousarray(inner_frame_distance, dtype=np.float32)
    contT = np.ascontiguousarray(containment, dtype=np.float32).T  # [B, T]
    tgt = np.ascontiguousarray(target).view(np.uint8)

    in_maps = _make_in_maps(iw, owd, ofd, ifd, contT, tgt)
    res = run_bass_kernel_spmd(nc, in_maps, list(range(NCORES)))

    # z[p, bc] (per core) = partial loss for b = bc*128 + p, summed over
    # the core's 8 towns.  Sum cores, flatten to [B], mean.
    acc = np.zeros((128, NBC), dtype=np.float64)
    for r in res.results:
        acc += r["z"].astype(np.float64)
    loss_b = acc.T.reshape(B)
    return np.float32(loss_b.mean())
